# revision 6
# baseline (speedup 1.0000x reference)
"""Low-rank Cayley linear kernel for TRN2 (8 NeuronCores, batch-sharded).

Math: reference computes W = (I+A) @ NS4(I-A) with A = U V^T - V U^T and
NS4 = 4 Newton-Schulz iterations for (I-A)^{-1} starting at X=I, which is
exactly the partial Neumann sum X4 = sum_{j=0}^{15} A^j.  With
C = [U, V] (n x 2r), D = [V, -U] (n x 2r) we have A = C D^T and
A^{j+1} = C E^j D^T where E = D^T C is (2r x 2r).  Therefore

    W = (I + A) X4 = I + C F D^T,   F = 2 * sum_{j=0}^{14} E^j + E^15

and the output is

    y = x @ W^T = x + (x @ D) @ (F^T C^T).

All the 2048^3 work collapses to two rank-128 GEMMs per token plus a
128x128 polynomial evaluated once (8x fewer FLOPs, matching headroom=8).

Per-core plan (core c gets batch element c; 8 tiles of 256 tokens):
  - setup: E-chain in f32/f32r (precision: bf16-everything measured 7e-3
    rel err vs 3e-3 for bf16-data-path-only; gate is 2e-2), then D and
    S = F^T C^T rounded to bf16 once.
  - stream x in 256-token tiles: SWDGE cast-DMA f32->bf16 on the gpsimd
    ring (halves SBUF traffic; HBM read unchanged), bf16 PE transposes
    (1 cyc/row + FWL weight loads, 2x faster than f32) grouped 8 per
    PSUM bank, flat ACT copy to x^T, stage1 P^T = D^T x^T (bf16, N=256),
    stage2 corr = P @ S (bf16, N=512), DVE add y = x + corr into an f32
    staging tile, 2 MB store on the sync HWDGE ring.  U/V load on the
    scalar HWDGE ring so nothing serializes ahead of the x-load stream.
"""

import numpy as np

import concourse.bacc as bacc
import concourse.bass as bass
import concourse.mybir as mybir
import concourse.tile as tile
from concourse.bass_utils import run_bass_kernel_spmd
from concourse.masks import make_identity

N = 2048          # model dim (N_IN == N_OUT)
R = 64            # rank of U, V
R2 = 2 * R        # 128
NCORES = 8
TOK = 2048        # tokens per core (one batch element)
F32 = mybir.dt.float32
F32R = mybir.dt.float32r
BF16 = mybir.dt.bfloat16
TILE = 256                 # tokens per main-loop tile
NSUB = TILE // 128         # 2 sub-blocks of 128 tokens
NTILE = TOK // TILE        # 8 tiles
NCHUNK = N // 128          # 16 feature chunks
NBLK = N // 512            # 4 output feature blocks
NGRP = (NCHUNK * NSUB) // 8  # 4 transpose groups of 8 blocks per tile

_NC_CACHE = {}


def _setup(nc, tc, ctx, u_d, v_d, const, setup, ps_s):
    """Emit weight construction; returns (ident_bf, D_bf, S_bf) persistent tiles.

    All scratch lives in the persistent `setup` pool: a scoped pool would be
    freed and re-used by the x tiles, which makes the very first x load wait
    on the entire setup chain (SBUF aliasing serializes the gpsimd FIFO).
    """
    ident = const.tile([128, 128], F32)
    make_identity(nc, ident[:])
    ident_bf = const.tile([128, 128], BF16)
    nc.vector.tensor_copy(out=ident_bf[:], in_=ident[:])
    D_bf = const.tile([128, NCHUNK, R2], BF16)
    S_bf = const.tile([128, N], BF16)

    if True:
        C_sb = setup.tile([128, NCHUNK, R2], F32)
        D_sb = setup.tile([128, NCHUNK, R2], F32)
        u_r = u_d[:].rearrange("(j p) r -> p j r", p=128)
        v_r = v_d[:].rearrange("(j p) r -> p j r", p=128)
        # scalar = ACT HWDGE ring: keeps these strided loads off the
        # gpsimd (x-load) and sync (y-store) rings.
        nc.scalar.dma_start(out=C_sb[:, :, 0:R], in_=u_r)
        nc.scalar.dma_start(out=C_sb[:, :, R:R2], in_=v_r)
        # D = [V, -U] built on-chip (free-dim slices, lane-local)
        nc.vector.tensor_copy(out=D_sb[:, :, 0:R], in_=C_sb[:, :, R:R2])
        nc.scalar.mul(D_sb[:, :, R:R2], C_sb[:, :, 0:R], -1.0)
        nc.vector.tensor_copy(out=D_bf[:], in_=D_sb[:])

        counter = [0]

        def fresh(tag=None):
            counter[0] += 1
            return setup.tile([128, 128], F32, name=f"sm{counter[0]}", tag=f"sm{counter[0]}")

        def accum_mm(lhs_view, rhs_view):
            ps = ps_s.tile([128, 512], F32, tag="small_mm")
            for j in range(NCHUNK):
                nc.tensor.matmul(
                    ps[:, 0:128],
                    lhs_view[:, j, :],
                    rhs_view[:, j, :],
                    start=(j == 0),
                    stop=(j == NCHUNK - 1),
                )
            out = fresh()
            nc.vector.tensor_copy(out=out[:], in_=ps[:, 0:128])
            return out

        def mm(lhsT, rhs):
            ps = ps_s.tile([128, 512], F32, tag="small_mm")
            nc.tensor.matmul(
                ps[:, 0:128], lhsT[:], rhs[:],
                start=True, stop=True,
            )
            out = fresh()
            nc.vector.tensor_copy(out=out[:], in_=ps[:, 0:128])
            return out

        def add_i(a):
            out = fresh()
            nc.vector.tensor_add(out=out[:], in0=ident[:], in1=a[:])
            return out

        E = accum_mm(D_sb, C_sb)       # E = D^T C
        ET = accum_mm(C_sb, D_sb)      # E^T = C^T D
        E2 = mm(ET, E)
        E2T = mm(E, ET)
        E3 = mm(E2T, E)
        E4 = mm(E2T, E2)
        E4T = mm(E2, E2T)
        E7 = mm(E4T, E3)
        E8 = mm(E4T, E4)
        E8T = mm(E4, E4T)
        E15 = mm(E8T, E7)
        A1T = add_i(ET)
        A2 = add_i(E2)
        A4 = add_i(E4)
        A8 = add_i(E8)
        T1T = mm(A2, A1T)
        T2T = mm(A4, T1T)
        S16 = mm(T2T, A8)
        F_sb = fresh()
        tmp2 = fresh()
        nc.vector.tensor_add(out=tmp2[:], in0=S16[:], in1=S16[:])
        nc.vector.tensor_sub(out=F_sb[:], in0=tmp2[:], in1=E15[:])

        # C^T via PE transposes
        CT = setup.tile([128, N], F32)
        for j in range(NCHUNK):
            ps = ps_s.tile([128, 512], F32, tag="small_mm")
            nc.tensor.transpose(ps[:, 0:128], C_sb[:, j, :], ident[:])
            nc.scalar.copy(out=CT[:, j * 128 : (j + 1) * 128], in_=ps[:, 0:128])

        # S = F^T C^T, rounded once to bf16
        for nblk in range(NBLK):
            ps = ps_s.tile([128, 512], F32, tag="small_mm")
            nc.tensor.matmul(
                ps[:],
                F_sb[:],
                CT[:, nblk * 512 : (nblk + 1) * 512],
                start=True, stop=True,
            )
            nc.scalar.copy(out=S_bf[:, nblk * 512 : (nblk + 1) * 512], in_=ps[:])

    return ident_bf, D_bf, S_bf


def _emit(nc, tc, ctx):
    x_d = nc.dram_tensor("x", [TOK, N], F32, kind="ExternalInput")
    u_d = nc.dram_tensor("u", [N, R], F32, kind="ExternalInput")
    v_d = nc.dram_tensor("v", [N, R], F32, kind="ExternalInput")
    y_d = nc.dram_tensor("y", [TOK, N], F32, kind="ExternalOutput")

    const = ctx.enter_context(tc.tile_pool(name="const", bufs=1))
    sconst = ctx.enter_context(tc.tile_pool(name="sscratch", bufs=1))
    ps_s = ctx.enter_context(tc.tile_pool(name="ps_s", bufs=2, space="PSUM"))
    xpool = ctx.enter_context(tc.tile_pool(name="xpool", bufs=4))
    xtpool = ctx.enter_context(tc.tile_pool(name="xtpool", bufs=2))
    ptpool = ctx.enter_context(tc.tile_pool(name="ptpool", bufs=2))
    ypool = ctx.enter_context(tc.tile_pool(name="ypool", bufs=2))
    ps_t = ctx.enter_context(tc.tile_pool(name="ps_t", bufs=2, space="PSUM"))
    ps_p = ctx.enter_context(tc.tile_pool(name="ps_p", bufs=2, space="PSUM"))
    ps_c = ctx.enter_context(tc.tile_pool(name="ps_c", bufs=2, space="PSUM"))

    x_r = x_d[:].rearrange("(t s p) f -> t p s f", p=128, s=NSUB)
    y_r = y_d[:].rearrange("(t s p) f -> t p s f", p=128, s=NSUB)

    x_tiles = {}
    pt_tiles = {}
    weights = {}  # filled by _setup; closures read at emission time

    def load(t):
        x_t = xpool.tile([128, NSUB, N], BF16, tag="x_t", name=f"x_t{t}")
        x_tiles[t] = x_t
        nc.gpsimd.dma_start(out=x_t[:], in_=x_r[t])  # SWDGE cast f32->bf16

    def head(t):
        """transpose tile t into xt, then stage1 -> pt."""
        x_t = x_tiles[t]
        ident_bf, D_bf = weights["ident_bf"], weights["D_bf"]
        xt = xtpool.tile([128, NCHUNK, TILE], BF16, tag="xt")
        for g in range(NGRP):
            ps = ps_t.tile([128, 1024], BF16, tag="ps_t")
            for jj in range(4):
                j = g * 4 + jj
                for i in range(NSUB):
                    b = jj * NSUB + i
                    nc.tensor.transpose(
                        ps[:, b * 128 : (b + 1) * 128],
                        x_t[:, i, j * 128 : (j + 1) * 128],
                        ident_bf[:],
                    )
            # psum block order (jj, i, q) == xt free layout (chunk, i*128+q)
            nc.scalar.copy(
                out=xt[:, g * 4 : (g + 1) * 4, :],
                in_=ps[:].rearrange("p (c q) -> p c q", c=4),
            )
        psp = ps_p.tile([128, TILE], F32, tag="ps_p")
        for j in range(NCHUNK):
            nc.tensor.matmul(
                psp[:],
                D_bf[:, j, :],
                xt[:, j, :],
                start=(j == 0),
                stop=(j == NCHUNK - 1),
            )
        pt = ptpool.tile([128, TILE], BF16, tag="pt")
        nc.scalar.copy(out=pt[:], in_=psp[:])
        pt_tiles[t] = pt

    def tail(t):
        """stage2 + add + store for tile t."""
        x_t = x_tiles[t]
        pt = pt_tiles[t]
        S_bf = weights["S_bf"]
        y_h = ypool.tile([128, NSUB, N], F32, tag="y_h")
        for i in range(NSUB):
            for nblk in range(NBLK):
                psc = ps_c.tile([128, 512], F32, tag="ps_c")
                nc.tensor.matmul(
                    psc[:],
                    pt[:, i * 128 : (i + 1) * 128],
                    S_bf[:, nblk * 512 : (nblk + 1) * 512],
                    start=True,
                    stop=True,
                )
                nc.vector.tensor_add(
                    out=y_h[:, i, nblk * 512 : (nblk + 1) * 512],
                    in0=psc[:],
                    in1=x_t[:, i, nblk * 512 : (nblk + 1) * 512],
                )
        nc.sync.dma_start(out=y_r[t], in_=y_h[:])

    # x loads first: the gpsimd FIFO must not queue anything that could
    # wait before the first loads' descriptor generation.
    load(0)
    load(1)
    load(2)

    ident_bf, D_bf, S_bf = _setup(nc, tc, ctx, u_d, v_d, const, sconst, ps_s)
    weights.update(ident_bf=ident_bf, D_bf=D_bf, S_bf=S_bf)

    for t in range(NTILE):
        if t + 3 < NTILE:
            load(t + 3)
        if t >= 1:
            tail(t - 1)
        head(t)
    tail(NTILE - 1)


def build_nc():
    key = ("v1",)
    if key in _NC_CACHE:
        return _NC_CACHE[key]
    nc = bacc.Bacc(
        "TRN2",
        target_bir_lowering=False,
        debug=False,
        enable_asserts=False,
        num_devices=NCORES,
    )
    from contextlib import ExitStack

    with tile.TileContext(nc) as tc, ExitStack() as ctx:
        _emit(nc, tc, ctx)
    nc.compile()
    _NC_CACHE[key] = nc
    return nc


def _run(input, U, V, trace=False):
    nc = build_nc()
    U = np.ascontiguousarray(U, dtype=np.float32)
    V = np.ascontiguousarray(V, dtype=np.float32)
    in_maps = [
        {"x": np.ascontiguousarray(input[c], dtype=np.float32), "u": U, "v": V}
        for c in range(NCORES)
    ]
    res = run_bass_kernel_spmd(nc, in_maps, list(range(NCORES)), trace=trace)
    out = np.stack([res.results[c]["y"] for c in range(NCORES)], axis=0)
    return out, res


def kernel(input, U, V):
    out, _ = _run(input, U, V, trace=False)
    return out
